# revision 1
# baseline (speedup 1.0000x reference)
"""MoE routing kernel for Trainium2, 8 NeuronCores, expert-parallel.

Strategy
--------
Host: gate (x @ Wg + bg), top-2 + softmax -> routing metadata only (0.025%
of the FLOPs); all expert-MLP compute and the combine run on device.

Expert-parallel: core c runs expert c on that expert's routed tokens
(gathered + transposed host-side, zero-padded to Cs, sorted by token id =
sorted by owner core). The per-expert activation is selected by DATA so
the SPMD program stays static:
    gelu(h) = h * (0.5 + 0.5*erf(h/sqrt(2)))     (even experts)
    silu(h) = h * sigmoid(h)                     (odd experts)
  t1 = Erf(s1*ps + bg)      even: s1=1/sqrt2, bg=b1/sqrt2   odd: 0,0 -> 0
  t2 = Sigmoid(s2*ps + bs)  even: 0,0 -> 0.5 (the gelu const!) odd: 1,b1
  aT = (ps + b1) * (0.5*t1 + t2)
Erf and Sigmoid share one ACT table set -> no table reloads.

mm1: h^T tile [F,tok] = W1tile^T @ xT   (W1 as lhsT, host-tiled layout)
mm2 (token-major): y[tok,d] = aT_chunk^T @ W2rows, accumulated over 4
d_ff quarters in SBUF; final (y + b2) * combine_weight.

Combine: permutation MATMULS instead of indirect DMA (all DMAs contiguous):
  send = Mperm^T @ y   (bucket-by-owner ordering, zero rows for pad slots)
  AllToAll send -> recv
  y_shard = M2^T @ recv  (two 1s per row: gathers AND adds the token's two
  expert contributions in one matmul)

Matmuls run in fp32r (full PE rate at moving-dim >= 256, ~1e-4 rel err).
"""

import numpy as np

D_MODEL, D_FF, N_EXPERTS, TOP_K = 1024, 4096, 8, 2
B, S = 2, 2048
T = B * S
NCORES = 8
P = 128
SHARD = T // NCORES     # 512 tokens owned per core
FD = D_FF // P          # 32 F-tiles
KD = D_MODEL // P       # 8 K-chunks (d_model)
DH = 2                  # d_model halves (N=512 matmul free dim)
FH = 4                  # d_ff quarters (SBUF residency of aT)
FHT = FD // FH          # 8 F-tiles per quarter

_prog_cache = {}
_wprep_cache = {}


def _chunks(Cs):
    """Token chunks, each >= 256 (fp32r full-rate) and <= 512."""
    assert Cs % 128 == 0 and Cs >= 256
    n = max(1, -(-Cs // 512))
    base = Cs // n
    base -= base % 128
    out = []
    o = 0
    for i in range(n):
        L = base if i < n - 1 else Cs - o
        out.append((o, L))
        o += L
    assert all(x[1] >= 256 or Cs < 256 + 128 for x in out)
    return out


def _build_program(Cs, CAP):
    import concourse.tile as tile
    from concourse import bacc, mybir
    import concourse.bass as bass

    f32 = mybir.dt.float32
    f32r = mybir.dt.float32r
    G = Cs // P          # token tiles
    SR = NCORES * CAP    # send/recv rows
    RC = SR // P         # recv-row tiles
    TG = SHARD // P      # owned-token tiles (4)
    CH = _chunks(Cs)
    assert RC <= KD + FHT, "recv tiles exceed reusable xt/aT slots"


    nc = bacc.Bacc("TRN2", target_bir_lowering=False, debug=False,
                   num_devices=NCORES)

    xT = nc.dram_tensor("xT", [D_MODEL, Cs], f32, kind="ExternalInput").ap()
    wcol = nc.dram_tensor("wcol", [Cs], f32, kind="ExternalInput").ap()
    # host-tiled weights: W1r [FD, 128, KD*128]; W2 natural rows [FD*128, 1024]
    W1r = nc.dram_tensor("W1r", [FD, P, KD * P], f32, kind="ExternalInput").ap()
    W2n = nc.dram_tensor("W2n", [D_FF, D_MODEL], f32, kind="ExternalInput").ap()
    aprm = nc.dram_tensor("aprm", [P, 2], f32, kind="ExternalInput").ap()  # s1, s2
    bgp = nc.dram_tensor("bgp", [P, FD], f32, kind="ExternalInput").ap()
    bsp = nc.dram_tensor("bsp", [P, FD], f32, kind="ExternalInput").ap()
    b1p = nc.dram_tensor("b1p", [P, FD], f32, kind="ExternalInput").ap()
    b2bc = nc.dram_tensor("b2bc", [P, D_MODEL], f32, kind="ExternalInput").ap()
    Mp = nc.dram_tensor("Mp", [SR // P, P, G * P], mybir.dt.uint8, kind="ExternalInput").ap()
    M2 = nc.dram_tensor("M2", [TG, P, RC * P], mybir.dt.uint8, kind="ExternalInput").ap()
    y_shard = nc.dram_tensor("y_shard", [SHARD, D_MODEL], f32,
                             kind="ExternalOutput").ap()

    send_buf = nc.dram_tensor("send_buf", [SR, D_MODEL], f32).ap()
    recv_buf = nc.dram_tensor("recv_buf", [SR, D_MODEL], f32).ap()

    with tile.TileContext(nc) as tc:
        with (
            tc.tile_pool(name="xtp", bufs=1) as xtp,
            tc.tile_pool(name="atp", bufs=1) as atp,
            tc.tile_pool(name="ytp", bufs=1) as ytp,
            tc.tile_pool(name="w1p", bufs=3) as w1p,
            tc.tile_pool(name="w2p", bufs=1) as w2p,
            tc.tile_pool(name="mpc", bufs=2) as mpc,
            tc.tile_pool(name="smalls", bufs=1) as smalls,
            tc.tile_pool(name="gsp", bufs=3) as gsp,
            tc.tile_pool(name="yrp", bufs=3) as yrp,
            tc.tile_pool(name="psm1", bufs=4, space="PSUM") as psm1,
            tc.tile_pool(name="psm2", bufs=4, space="PSUM") as psm2,
        ):
            xts = []
            for k in range(KD):
                xt = xtp.tile([P, Cs], f32r, tag=f"xt{k}", name=f"xt{k}")
                nc.sync.dma_start(out=xt[:],
                                  in_=xT[k * P:(k + 1) * P, :].bitcast(f32r))
                xts.append(xt)

            asc = smalls.tile([P, 2], f32, tag="asc")
            nc.sync.dma_start(out=asc[:], in_=aprm[:, :])
            bgt = smalls.tile([P, FD], f32, tag="bgt")
            nc.sync.dma_start(out=bgt[:], in_=bgp[:, :])
            bst = smalls.tile([P, FD], f32, tag="bst")
            nc.sync.dma_start(out=bst[:], in_=bsp[:, :])
            b1t = smalls.tile([P, FD], f32, tag="b1t")
            nc.sync.dma_start(out=b1t[:], in_=b1p[:, :])
            b2t = smalls.tile([P, D_MODEL], f32, tag="b2t")
            nc.sync.dma_start(out=b2t[:], in_=b2bc[:, :])
            wct = smalls.tile([P, G], f32, tag="wct")
            nc.sync.dma_start(out=wct[:], in_=wcol.rearrange("(g p) -> p g", p=P))

            ytm = []
            for g in range(G):
                y = ytp.tile([P, D_MODEL], f32r, tag=f"ytm{g}", name=f"ytm{g}")
                ytm.append(y)

            for fh in range(FH):
                # ---- mm1 + activation -> aT (fp32r), this d_ff quarter
                aT = []
                for f in range(FHT):
                    a = atp.tile([P, Cs], f32r, tag=f"aT{f}", name=f"aT{f}")
                    aT.append(a)
                for f in range(FHT):
                    fg = fh * FHT + f
                    w1f = w1p.tile([P, KD * P], f32r, tag="w1f")
                    nc.sync.dma_start(out=w1f[:], in_=W1r[fg].bitcast(f32r))
                    for (o, L) in CH:
                        ps = psm1.tile([P, 512], mybir.dt.float32, tag="psm1")
                        for k in range(KD):
                            nc.tensor.matmul(ps[:, :L],
                                             lhsT=w1f[:, k * P:(k + 1) * P],
                                             rhs=xts[k][:, o:o + L],
                                             start=(k == 0), stop=(k == KD - 1))
                        t1 = gsp.tile([P, 512], f32, tag="t1")
                        t2 = gsp.tile([P, 512], f32, tag="t2")
                        nc.scalar.activation(
                            t1[:, :L], ps[:, :L],
                            mybir.ActivationFunctionType.Erf,
                            bias=bgt[:, fg:fg + 1], scale=asc[:, 0:1])
                        nc.scalar.activation(
                            t2[:, :L], ps[:, :L],
                            mybir.ActivationFunctionType.Sigmoid,
                            bias=bst[:, fg:fg + 1], scale=asc[:, 1:2])
                        # v = 0.5*t1 + t2 ; aT = (ps + b1) * v
                        nc.vector.scalar_tensor_tensor(
                            out=t1[:, :L], in0=t1[:, :L], scalar=0.5,
                            in1=t2[:, :L],
                            op0=mybir.AluOpType.mult, op1=mybir.AluOpType.add)
                        nc.vector.scalar_tensor_tensor(
                            out=aT[f][:, o:o + L], in0=ps[:, :L],
                            scalar=b1t[:, fg:fg + 1], in1=t1[:, :L],
                            op0=mybir.AluOpType.add, op1=mybir.AluOpType.mult)

                # ---- mm2 (token-major) partial over this F-quarter
                w2ks = []
                for k in range(FHT):
                    kg = fh * FHT + k
                    w2k = w2p.tile([P, D_MODEL], f32r, tag=f"w2k{k}",
                                   name=f"w2k{k}")
                    nc.sync.dma_start(out=w2k[:],
                                      in_=W2n[kg * P:(kg + 1) * P, :].bitcast(f32r))
                    w2ks.append(w2k)
                for g in range(G):
                    for dh in range(DH):
                        ps = psm2.tile([P, 512], mybir.dt.float32, tag="psm2")
                        for k in range(FHT):
                            nc.tensor.matmul(
                                ps[:],
                                lhsT=aT[k][:, g * P:(g + 1) * P],
                                rhs=w2ks[k][:, dh * 512:(dh + 1) * 512],
                                start=(k == 0), stop=(k == FHT - 1))
                        ysl = ytm[g][:, dh * 512:(dh + 1) * 512]
                        if fh == 0:
                            nc.vector.tensor_add(
                                ysl, ps[:], b2t[:, dh * 512:(dh + 1) * 512])
                        elif fh < FH - 1:
                            nc.vector.tensor_add(ysl, ysl, ps[:])
                        else:
                            nc.vector.tensor_add(ysl, ysl, ps[:])
                            nc.vector.tensor_scalar_mul(ysl, ysl, wct[:, g:g + 1])

            # ---- sender permutation matmul -> send_buf (bucket order)
            for sr in range(SR // P):
                mpu = mpc.tile([P, G * P], mybir.dt.uint8, tag="mpu", name="mpu")
                nc.sync.dma_start(out=mpu[:], in_=Mp[sr])
                mpt = mpc.tile([P, G * P], f32r, tag="mpt", name="mpt")
                nc.vector.tensor_copy(mpt[:], mpu[:])
                for dh in range(DH):
                    ps = psm2.tile([P, 512], mybir.dt.float32, tag="psm2")
                    for g in range(G):
                        nc.tensor.matmul(
                            ps[:],
                            lhsT=mpt[:, g * P:(g + 1) * P],
                            rhs=ytm[g][:, dh * 512:(dh + 1) * 512],
                            start=(g == 0), stop=(g == G - 1))
                    st = yrp.tile([P, 512], f32, tag="st")
                    nc.vector.tensor_copy(st[:], ps[:])
                    nc.sync.dma_start(
                        out=send_buf[sr * P:(sr + 1) * P,
                                     dh * 512:(dh + 1) * 512],
                        in_=st[:])

            nc.gpsimd.collective_compute(
                "AllToAll",
                mybir.AluOpType.bypass,
                replica_groups=[list(range(NCORES))],
                ins=[send_buf[:, :]],
                outs=[recv_buf[:, :]],
            )

            # ---- owner combine matmul: y_shard = M2^T @ recv
            # recv tiles reuse the (now idle) xt/aT pool slots
            rcvs = []
            for rc in range(RC):
                if rc < KD:
                    rt = xtp.tile([P, D_MODEL], f32r, tag=f"xt{rc}",
                                  name=f"rcv{rc}")
                else:
                    rt = atp.tile([P, D_MODEL], f32r, tag=f"aT{rc - KD}",
                                  name=f"rcv{rc}")
                nc.sync.dma_start(out=rt[:],
                                  in_=recv_buf[rc * P:(rc + 1) * P, :].bitcast(f32r))
                rcvs.append(rt)
            for tg in range(TG):
                m2u = mpc.tile([P, RC * P], mybir.dt.uint8, tag="m2u", name="m2u")
                nc.sync.dma_start(out=m2u[:], in_=M2[tg])
                m2t = mpc.tile([P, RC * P], f32r, tag="m2t", name="m2t")
                nc.vector.tensor_copy(m2t[:], m2u[:])
                for dh in range(DH):
                    ps = psm2.tile([P, 512], mybir.dt.float32, tag="psm2")
                    for rc in range(RC):
                        nc.tensor.matmul(
                            ps[:],
                            lhsT=m2t[:, rc * P:(rc + 1) * P],
                            rhs=rcvs[rc][:, dh * 512:(dh + 1) * 512],
                            start=(rc == 0), stop=(rc == RC - 1))
                    ot = yrp.tile([P, 512], f32, tag="ot")
                    nc.vector.tensor_copy(ot[:], ps[:])
                    nc.sync.dma_start(
                        out=y_shard[tg * P:(tg + 1) * P,
                                    dh * 512:(dh + 1) * 512],
                        in_=ot[:])

    nc.compile()
    return nc


def _route(x_flat, Wg, bg):
    logits = x_flat.astype(np.float32) @ Wg.astype(np.float32) + bg
    order = np.argsort(-logits, axis=1, kind="stable")
    i1, i2 = order[:, 0], order[:, 1]
    s1 = np.take_along_axis(logits, i1[:, None], 1)[:, 0]
    s2 = np.take_along_axis(logits, i2[:, None], 1)[:, 0]
    e = np.exp((s2 - s1).astype(np.float32))
    w1 = 1.0 / (1.0 + e)
    w2 = e * w1
    return i1, i2, w1.astype(np.float32), w2.astype(np.float32)


def _prep_weights(W1):
    key = id(W1)
    hit = _wprep_cache.get(key)
    if hit is not None:
        return hit
    W1 = np.asarray(W1, np.float32)
    # W1r[e, f, p, k*128+q] = W1[e, k*128+p, f*128+q]
    W1r = np.ascontiguousarray(
        W1.reshape(N_EXPERTS, KD, P, FD, P).transpose(0, 3, 2, 1, 4)
        .reshape(N_EXPERTS, FD, P, KD * P))
    _wprep_cache.clear()
    _wprep_cache[key] = W1r
    return W1r


def make_in_maps(x, W1, b1, W2, b2, Wg, bg):
    """Single-pass maps (used by tests); asserts the pass covers everything."""
    x_flat, jobs, consts = _prepare(x, W1, b1, W2, b2, Wg, bg)
    return _pass_maps(x_flat, jobs, consts, strict=True)


MAX_CS = 1152


def _prepare(x, W1, b1, W2, b2, Wg, bg):
    x = np.asarray(x, np.float32)
    W2 = np.asarray(W2, np.float32)
    b1 = np.asarray(b1, np.float32)
    b2 = np.asarray(b2, np.float32)
    x_flat = np.ascontiguousarray(x.reshape(T, D_MODEL))
    i1, i2, w1, w2 = _route(x_flat, np.asarray(Wg, np.float32),
                            np.asarray(bg, np.float32))
    W1r = _prep_weights(W1)

    jobs = {}  # expert -> (ids, wts)   (ids ascending = sorted by owner)
    for e in range(N_EXPERTS):
        sel = (i1 == e) | (i2 == e)
        ids = np.nonzero(sel)[0]
        wts = np.where(i1[ids] == e, w1[ids], w2[ids]).astype(np.float32)
        jobs[e] = (ids, wts)
    return x_flat, jobs, (W1r, W2, b1, b2)


def _pass_maps(x_flat, jobs, consts, strict=False):
    W1r, W2, b1, b2 = consts
    maxlen = max(len(v[0]) for v in jobs.values())
    Cs = max(256, -(-maxlen // 128) * 128)
    assert Cs <= MAX_CS
    G = Cs // P

    bucket_count = np.zeros((NCORES, NCORES), np.int64)
    for e in range(NCORES):
        ids, _ = jobs[e]
        own = ids // SHARD
        for o in range(NCORES):
            bucket_count[e, o] += (own == o).sum()
    CAP = max(16, int(-(-bucket_count.max() // 16) * 16))
    SR = NCORES * CAP
    RC = SR // P
    TG = SHARD // P

    # send rows (per sender) + recv rows (per owner) for each contribution
    src_rows = np.full((T, 2), -1, np.int64)
    sr_of = {}
    for e in range(NCORES):
        ids, _ = jobs[e]
        own = ids // SHARD
        ks = np.empty(len(ids), np.int64)
        fill = np.zeros(NCORES, np.int64)
        for o in range(NCORES):
            m = own == o
            n = int(m.sum())
            ks[m] = fill[o] + np.arange(n)
            fill[o] += n
        sr_of[e] = own * CAP + ks
        rows_recv = e * CAP + ks
        which = (src_rows[ids, 0] >= 0).astype(np.int64)
        src_rows[ids, which] = rows_recv
    if strict:
        assert (src_rows >= 0).all()

    sq2 = np.float32(1.0 / np.sqrt(2.0))
    in_maps = []
    for c in range(NCORES):
        e = c
        ids, wts = jobs[e]
        L = len(ids)
        xTc = np.zeros((D_MODEL, Cs), np.float32)
        wcol = np.zeros(Cs, np.float32)
        if L:
            xTc[:, :L] = x_flat[ids].T
            wcol[:L] = wts
        even = (e % 2 == 0)
        b1_cols = np.ascontiguousarray(b1[e].reshape(FD, P).T)  # [P, FD]
        aprm = np.zeros((P, 2), np.float32)
        if even:
            aprm[:, 0] = sq2
            bgp = b1_cols * sq2
            bsp = np.zeros((P, FD), np.float32)
        else:
            aprm[:, 1] = 1.0
            bgp = np.zeros((P, FD), np.float32)
            bsp = b1_cols
        # packed layouts: Mp[sr_tile, p, g*128+q]: token g*128+p -> send row
        # sr_tile*128+q ; M2[tg, p, rc*128+q]: recv row rc*128+p -> token
        # tg*128+q
        Mp = np.zeros((SR // P, P, G * P), np.uint8)
        j = np.arange(L)
        srj = sr_of[e]
        Mp[srj // P, j % P, (j // P) * P + srj % P] = 1
        M2 = np.zeros((TG, P, RC * P), np.uint8)
        tok0 = c * SHARD
        tl = np.arange(SHARD)
        for col in range(2):
            r = src_rows[tok0:tok0 + SHARD, col]
            m = r >= 0
            M2[tl[m] // P, r[m] % P, (r[m] // P) * P + tl[m] % P] += 1
        in_maps.append({
            "xT": xTc, "wcol": wcol,
            "W1r": W1r[e], "W2n": W2[e],
            "aprm": aprm, "bgp": np.ascontiguousarray(bgp),
            "bsp": np.ascontiguousarray(bsp), "b1p": b1_cols,
            "b2bc": np.ascontiguousarray(np.broadcast_to(b2[e], (P, D_MODEL))),
            "Mp": Mp, "M2": M2,
        })
    return (Cs, CAP), in_maps


def get_program(key):
    if key not in _prog_cache:
        _prog_cache[key] = _build_program(*key)
    return _prog_cache[key]


def kernel(x, W1, b1, W2, b2, Wg, bg):
    from concourse.bass_utils import run_bass_kernel_spmd

    x_flat, jobs, consts = _prepare(x, W1, b1, W2, b2, Wg, bg)
    maxlen = max(len(v[0]) for v in jobs.values())
    npass = max(1, -(-maxlen // MAX_CS))
    out = None
    for p in range(npass):
        jobs_p = {e: (ids[p::npass], wts[p::npass])
                  for e, (ids, wts) in jobs.items()}
        key, in_maps = _pass_maps(x_flat, jobs_p, consts, strict=(npass == 1))
        nc = get_program(key)
        res = run_bass_kernel_spmd(nc, in_maps, list(range(NCORES)))
        full = np.concatenate(
            [res.results[c]["y_shard"] for c in range(NCORES)], axis=0)
        out = full if out is None else out + full
    return np.ascontiguousarray(out.reshape(B, S, D_MODEL))

